# revision 7
# baseline (speedup 1.0000x reference)
"""Trainium2 Bass kernel: two-layer LIF spiking network scan.

Model (per timestep t, batch row b):
    h1 = x_t @ W1.T + b1            # [B, 32]
    v1 = v1 + (h1 - v1)/2           # tau = 2
    s1 = (v1 >= 1);  v1 *= (1-s1)   # hard reset
    h2 = s1 @ W2.T + b2             # [B, 1]
    v2 = v2 + (h2 - v2)/2
    s2 = (v2 >= 1);  v2 *= (1-s2)
    out = sum of s2 over t in [T - T//4, T)

Fast path (time-sharded, weight-specialized):
  * Only neurons h with max(W1[h,0],0)+max(W1[h,1],0) >= VTH can ever spike
    (x in [0,1]); the rest are dead for any input and are pruned exactly.
  * If sum_h max(W2[0,h],0) over live h < 1, layer 2 provably never resets,
    so v2 is a pure EMA: v2_t = 0.5 v2_{t-1} + 0.5 h2_t.
  * The LIF decay (tau=2) forgets exponentially: starting a window K steps
    early from v=0 reproduces the true state to ~2^-K.  With K=64 the scheme
    is exact far below fp32 noise, so the 8 cores each own a 512-step time
    window of the FULL batch (g0 = c*512 - 64) instead of a batch shard.
  * Engine split per core: PE computes c = 0.5*h1 for live neurons via
    block-diagonal matmuls (bf16, one [64,128]x[64,160] per step); ACT stages
    PSUM->SBUF (bf16); DVE runs the serial LIF step op plus chunked spike
    contributions and the v2 EMA scan; ACT counts decision-window spikes via
    a Sign-accumulate.  Host sums the two cores whose windows lie in the
    decision region.

Anything outside the envelope (weights with >8 live neurons, possible layer-2
resets, x outside [0,1], nonzero biases) falls back to the original
batch-parallel per-step kernel below, which handles the general case.
"""

import numpy as np

B, T, I, H, O = 4096, 4096, 2, 32, 1
N_CORES = 8
B_CORE = B // N_CORES          # 512 (fallback path)
G = B_CORE // 128              # 4 groups (fallback path)
TAU_INV = 0.5
VTH = 1.0

_cache = {}


# ----------------------------------------------------------------- custom ops
def _register_custom_ops():
    """Register our custom DVE ops in the process-global registry (idempotent)."""
    import concourse.dve_ops as dve_ops_mod
    from concourse.dve_ops import DveOp
    from concourse.dve_spec import (
        Spec, Src0, Src1, C0, C1, C2, Zero, One,
        select, eq, lower, AluOp, scan, _has_src1,
    )
    from concourse.dve_uop import DveOpSpec

    if "ANT_SNN_FMA2" in dve_ops_mod._SUB_OPCODE_FOR_NAME:
        return

    def _ref_fma2(in0, in1, s0, s1, imm2):
        return (in0 * s0 + in1 * s1).astype(np.float32)

    def _ref_lif1(in0, in1, s0, s1, imm2):
        # state is the pre-reset potential u: u' = (u<1) ? 0.5u + c : c
        return np.where(
            in0 < 1.0, (in0 * np.float32(0.5)) + in1, in1
        ).astype(np.float32)

    def _ref_sds(in0, in1, s0, s1, imm2):
        # prefix sums of (u >= 1) * w2h along the free dim
        contrib = np.where(in0 < 1.0, np.float32(0.0), in1)
        return np.cumsum(contrib.astype(np.float32), axis=-1, dtype=np.float32)

    specs = [
        ("ANT_SNN_FMA2", Spec(body=Src0 * C0 + Src1 * C1, reference=_ref_fma2)),
        (
            "ANT_SNN_LIF1",
            Spec(
                body=select(Src0 < One, Src0 * C0 + Src1, Src1),
                reference=_ref_lif1,
            ),
        ),
        (
            "ANT_SNN_SDS",
            Spec(
                body=scan(AluOp.ADD, select(Src0 < One, Zero, Src1)),
                reference=_ref_sds,
            ),
        ),
    ]

    ops = {}
    for name, spec in specs:
        row = 1 + len(dve_ops_mod.OPS)
        sha = {}
        for ver in ("v3", "v4"):
            try:
                s = DveOpSpec(
                    name=name,
                    opcode=row,
                    uops=lower(spec, ver=ver),
                    rd1_en=_has_src1(spec),
                )
                sha[ver] = s.sha(ver)
            except Exception:
                pass
        op = DveOp(name, spec, subdim=False, uops_sha=sha)
        dve_ops_mod.OPS.append(op)
        dve_ops_mod.CUSTOM_DVE_SPECS[name] = spec
        dve_ops_mod._SUB_OPCODE_FOR_NAME[name] = row
        ops[name] = op
    return ops


def _get_ops():
    import concourse.dve_ops as dve_ops_mod

    _register_custom_ops()
    by_name = {op.name: op for op in dve_ops_mod.OPS}
    return (
        by_name["ANT_SNN_FMA2"],
        by_name["ANT_SNN_LIF1"],
        by_name["ANT_SNN_SDS"],
    )


# =================================================================== fast path
WARM = 64                     # warmup steps before each core's 512-step window
WS = 512 + WARM               # steps simulated per core
CH = 48                       # DVE/d-chain chunk (steps); WS % CH == 0
NCH = WS // CH                # 12 chunks
RING = 2 * CH                 # uring/cring slots (96)
BGF = 32                      # batch groups of 128 over the FULL batch


def build_fast(HA):
    """Per-core Bass program for the time-sharded fast path.

    HA = number of live hidden neurons (spb = PSUM steps per bank must be >=1).
    """
    import concourse.bass as bass
    import concourse.mybir as mybir

    _, OP_LIF, _ = _get_ops()
    A = mybir.AluOpType
    f32 = mybir.dt.float32
    bf16 = mybir.dt.bfloat16
    Sign = mybir.ActivationFunctionType.Sign

    NW = HA * BGF                      # matmul N / per-step state width
    spb = max(1, 512 // NW)            # matmul outputs per PSUM bank
    spp = 2 * spb                      # steps per ACT copy pair (two banks)
    assert RING % spp == 0 and WS % spp == 0

    nc = bass.Bass(detect_race_conditions=False)

    xstat = nc.declare_dram_parameter("xstat", [2 * BGF, WS * 128], bf16, isOutput=False)
    wmov = nc.declare_dram_parameter("wmov", [2 * BGF, NW], bf16, isOutput=False)
    w2s = nc.declare_dram_parameter("w2s", [128, HA], f32, isOutput=False)
    wselh = nc.declare_dram_parameter("wselh", [128, 1], f32, isOutput=False)
    out = nc.declare_dram_parameter("out", [128, BGF], f32, isOutput=True)

    XCH = CH * 128
    xs_sb = nc.alloc_sbuf_tensor("xs_sb", [2 * BGF, 2, XCH], bf16)
    wmov_sb = nc.alloc_sbuf_tensor("wmov_sb", [2 * BGF, NW], bf16)
    w2s_sb = nc.alloc_sbuf_tensor("w2s_sb", [128, HA], f32)
    wselh_sb = nc.alloc_sbuf_tensor("wselh_sb", [128, 1], f32)
    uring = nc.alloc_sbuf_tensor("uring", [128, RING, NW], bf16)
    # cring rows mirror the PSUM bank pairs so one ACT copy's in/out shapes
    # match: [pair_slot, bank_of_pair, spb*NW]
    cring = nc.alloc_sbuf_tensor(
        "cring", [128, RING // (2 * spb), 2, spb * NW], bf16
    )
    dA = nc.alloc_sbuf_tensor("dA", [128, CH, BGF], bf16)
    dB = nc.alloc_sbuf_tensor("dB", [128, CH, BGF], bf16)
    k2r = nc.alloc_sbuf_tensor("k2r", [128, WS, BGF], bf16)
    u2r = nc.alloc_sbuf_tensor("u2r", [128, BGF, WS], bf16)
    halves = nc.alloc_sbuf_tensor("halves", [128, WS], bf16)
    nbias = nc.alloc_sbuf_tensor("nbias", [128, 1], f32)
    sgn = nc.alloc_sbuf_tensor("sgn", [128, 512], bf16)
    accs = nc.alloc_sbuf_tensor("accs", [128, BGF], f32)
    counts = nc.alloc_sbuf_tensor("counts", [128, BGF], f32)
    psum = nc.alloc_psum_tensor("ps", [128, 8, 512], f32)

    NPAIR = WS // spp                  # ACT copy pairs

    with (
        nc.semaphore("s_w") as s_w,
        nc.semaphore("s_pe") as s_pe,
        nc.semaphore("s_act") as s_act,
        nc.semaphore("s_dvec") as s_dvec,
        nc.semaphore("s_pex") as s_pex,
        nc.semaphore("s_step") as s_step,
        nc.semaphore("s_u2") as s_u2,
        nc.semaphore("s_cnt") as s_cnt,
        nc.semaphore("s_fix") as s_fix,
        nc.semaphore("s_out") as s_out,
        nc.Block() as block,
    ):
        sem_x = [nc.semaphore(f"sx{k}").__enter__() for k in range(NCH)]

        @block.sync
        def _(sync):
            sync.dma_start(out=wmov_sb.ap()[:, :], in_=wmov[:, :]).then_inc(s_w, 16)
            sync.dma_start(out=w2s_sb.ap()[:, :], in_=w2s[:, :]).then_inc(s_w, 16)
            sync.dma_start(out=wselh_sb.ap()[:, :], in_=wselh[:, :]).then_inc(s_w, 16)
            for k in range(NCH):
                if k >= 2:
                    # xs_sb slot reuse: PE must be done with chunk k-2
                    sync.wait_ge(s_pex, k - 1)
                sync.dma_start(
                    out=xs_sb.ap()[:, k % 2, :],
                    in_=xstat[:, k * XCH:(k + 1) * XCH],
                ).then_inc(sem_x[k], 16)
            sync.wait_ge(s_fix, 1)
            sync.dma_start(out=out[:, :], in_=counts.ap()[:, :]).then_inc(s_out, 16)
            sync.wait_ge(s_out, 16)

        @block.tensor
        def _(tensor):
            tensor.wait_ge(s_w, 16)
            for t in range(WS):
                k = t // CH
                if t % CH == 0:
                    tensor.wait_ge(sem_x[k], 16)
                g = t // spb                      # bank group index
                if t % spb == 0 and g >= 8:
                    # bank reuse: ACT pair (g-8)//2 must have drained it
                    tensor.wait_ge(s_act, (g - 8) // 2 + 1)
                ins = tensor.matmul(
                    psum.ap()[:, g % 8, (t % spb) * NW:(t % spb) * NW + NW],
                    xs_sb.ap()[:, k % 2, (t % CH) * 128:(t % CH) * 128 + 128],
                    wmov_sb.ap()[:, :],
                    True, True,
                )
                if t % spb == spb - 1:
                    ins.then_inc(s_pe, 1)
                if t % CH == CH - 1:
                    ins.then_inc(s_pex, 1)

        @block.scalar
        def _(scalar):
            for p in range(NPAIR):
                t0 = p * spp
                kc = t0 // CH
                scalar.wait_ge(s_pe, 2 * p + 2)
                if kc >= 2:
                    # cring slot reuse: DVE must be done with chunk kc-2
                    scalar.wait_ge(s_dvec, kc - 1)
                scalar.copy(
                    out=cring.ap()[:, (t0 // spp) % (RING // spp), :, :],
                    in_=psum.ap()[:, (2 * p) % 8:(2 * p) % 8 + 2, 0:spb * NW],
                ).then_inc(s_act, 1)
            # --- decision-window spike count (per batch group) ---
            scalar.wait_ge(s_u2, 1)
            for gidx in range(BGF):
                ins = scalar.activation(
                    out=sgn.ap()[:, :],
                    in_=u2r.ap()[:, gidx, WARM:WS],
                    func=Sign,
                    bias=nbias.ap()[:, :],
                    scale=1.0,
                    accum_out=accs.ap()[:, gidx:gidx + 1],
                )
                if gidx == BGF - 1:
                    ins.then_inc(s_cnt, 1)

        @block.vector
        def _(vector):
            vector.memset(uring.ap()[:, RING - 1, :], 0.0)
            vector.memset(halves.ap()[:, :], 0.5)
            vector.memset(nbias.ap()[:, :], -0.998)
            vector.wait_ge(s_w, 48)
            for t in range(WS):
                if t % spp == 0:
                    vector.wait_ge(s_act, t // spp + 1)
                # serial LIF step over all live chains; the then_inc flushes
                # the write so the next op's read is fresh in the interp.
                ins = vector._custom_dve(
                    OP_LIF,
                    out=uring.ap()[:, t % RING, :],
                    in0=uring.ap()[:, (t - 1) % RING, :],
                    in1=cring.ap()[
                        :, (t // spp) % (RING // spp), (t // spb) % 2,
                        (t % spb) * NW:(t % spb) * NW + NW,
                    ],
                    s0=0.5,
                )
                if t % CH == CH - 1:
                    # cring chunk fully consumed -> ACT may reuse the slots
                    # (this inc also flushes the state write for the d-chain)
                    ins.then_inc(s_dvec, 1)
                    kc = t // CH
                    sl = (kc % 2) * CH
                    u3 = uring.ap()
                    tsl = slice(sl, sl + CH)
                    vector.tensor_scalar(
                        dA.ap()[:, :, :], u3[:, tsl, 0:BGF],
                        1.0, w2s_sb.ap()[:, 0:1], A.is_ge, A.mult,
                    )
                    for h in range(1, HA):
                        vector.tensor_scalar(
                            dB.ap()[:, :, :], u3[:, tsl, h * BGF:(h + 1) * BGF],
                            1.0, w2s_sb.ap()[:, h:h + 1], A.is_ge, A.mult,
                        ).then_inc(s_step, 1)
                        tgt = (
                            k2r.ap()[:, kc * CH:(kc + 1) * CH, :]
                            if h == HA - 1 else dA.ap()[:, :, :]
                        )
                        vector.tensor_tensor(
                            out=tgt, in0=dA.ap()[:, :, :], in1=dB.ap()[:, :, :],
                            op=A.add,
                        ).then_inc(s_step, 1)
                else:
                    ins.then_inc(s_step, 1)
            # --- v2 EMA per batch group (never resets: sup u2 < 1) ---
            for gidx in range(BGF):
                ins = vector.tensor_tensor_scan(
                    u2r.ap()[:, gidx, :],
                    halves.ap()[:, :],
                    k2r.ap()[:, :, gidx],
                    0.0, A.mult, A.add,
                )
                if gidx == BGF - 1:
                    ins.then_inc(s_u2, 1)
            vector.wait_ge(s_cnt, 1)
            vector.tensor_scalar(
                counts.ap()[:, :], accs.ap()[:, :], 512.0, wselh_sb.ap()[:, :],
                A.add, A.mult,
            ).then_inc(s_fix, 1)

    mybir.codegen_inst_isa_subclasses(nc)
    return nc


def _fast_envelope(x, W1, b1, W2, b2):
    """Live-neuron set + rigorous layer-2-silence check.  Returns the live
    index array, or None if the fast path's assumptions don't hold."""
    if b1.any() or b2.any():
        return None
    if x.min() < 0.0 or x.max() > 1.0:
        return None
    sup1 = np.maximum(W1[:, 0], 0) + np.maximum(W1[:, 1], 0)
    live = np.where(sup1 >= VTH)[0]
    if len(live) < 1 or len(live) > 8:
        return None
    # u2 <= sum_h w2h+ for any spike pattern; need strict margin for bf16 slop
    if np.maximum(W2[0, live], 0).sum() >= 0.95:
        return None
    return live


def _fast_inputs(x, W1, W2, live):
    import jax.numpy as jnp

    HA = len(live)
    NW = HA * BGF
    to_bf = lambda a: np.asarray(jnp.asarray(np.asarray(a, np.float32), jnp.bfloat16))

    # wmov[(bg,i), h*BGF+bg] = 0.5*W1[live[h], i]
    wmov = np.zeros((2 * BGF, NW), np.float32)
    for bg in range(BGF):
        for i in range(I):
            for h in range(HA):
                wmov[bg * 2 + i, h * BGF + bg] = 0.5 * W1[live[h], i]
    wmov = to_bf(wmov)
    w2s = np.tile((0.5 * W2[0, live])[None, :], (128, 1)).astype(np.float32)

    # xstat per core: [(bg,i), t*128+p] = x[bg*128+p, g0+t, i]
    xr = x.reshape(BGF, 128, T, I)                    # [bg, p, t, i]
    xstats = []
    for c in range(N_CORES):
        g0 = c * 512 - WARM
        arr = np.zeros((BGF, 128, WS, I), np.float32)
        lo, hi = max(g0, 0), min(g0 + WS, T)
        arr[:, :, lo - g0:hi - g0, :] = xr[:, :, lo:hi, :]
        # -> [(bg, i), t, p]
        arr = arr.transpose(0, 3, 2, 1).reshape(2 * BGF, WS * 128)
        xstats.append(to_bf(arr))

    in_maps = []
    for c in range(N_CORES):
        wsel = 0.5 if c * 512 >= max(T - T // 4, T // 2) else 0.0
        in_maps.append({
            "xstat": xstats[c],
            "wmov": wmov,
            "w2s": w2s,
            "wselh": np.full((128, 1), wsel, np.float32),
        })
    return in_maps


# ------------------------------------------------------------- fallback build
def build_nc(t_steps=T, decision_start=None, has_b1=False, has_b2=False):
    """Build the per-core Bass program (SPMD; all cores run the same NEFF)."""
    import concourse.bass as bass
    import concourse.mybir as mybir

    OP_FMA2, OP_LIF1, OP_SDS = _get_ops()
    A = mybir.AluOpType
    f32 = mybir.dt.float32

    if decision_start is None:
        decision_start = max(t_steps - t_steps // 4, t_steps // 2)

    # Same-engine RAW hazards are safe on HW (per-op DVE pipeline drain);
    # the CoreSim race detector would flag them, so turn it off.
    nc = bass.Bass(detect_race_conditions=False)

    xs = nc.declare_dram_parameter("xs", [B_CORE, t_steps * I], f32, isOutput=False)
    wc0b = nc.declare_dram_parameter("wc0b", [128, H], f32, isOutput=False)
    wc1b = nc.declare_dram_parameter("wc1b", [128, H], f32, isOutput=False)
    w2hb = nc.declare_dram_parameter("w2hb", [128, G * H], f32, isOutput=False)
    k2b = nc.declare_dram_parameter("k2b", [128, 1], f32, isOutput=False)
    b1hb = nc.declare_dram_parameter("b1hb", [128, G * H], f32, isOutput=False)
    out = nc.declare_dram_parameter("out", [128, G], f32, isOutput=True)

    xlen = t_steps * I
    FW = G * H  # 128 free width for the fused tiles

    x_sbuf = nc.alloc_sbuf_tensor("x_sbuf", [128, G * xlen], f32).ap()
    wc0 = nc.alloc_sbuf_tensor("wc0", [128, H], f32).ap()
    wc1 = nc.alloc_sbuf_tensor("wc1", [128, H], f32).ap()
    w2h = nc.alloc_sbuf_tensor("w2h", [128, FW], f32).ap()
    b1h = nc.alloc_sbuf_tensor("b1h", [128, FW], f32).ap()
    k2 = nc.alloc_sbuf_tensor("k2", [128, 1], f32).ap()
    NS = 8  # scan ring depth (DVE->gpsimd decoupling, in steps)
    SW = FW + 4  # scan slot width
    S0 = nc.alloc_sbuf_tensor("S0", [128, FW], f32).ap()
    S1 = nc.alloc_sbuf_tensor("S1", [128, FW], f32).ap()
    cbuf = nc.alloc_sbuf_tensor("cbuf", [128, FW], f32).ap()
    scanring = nc.alloc_sbuf_tensor("scanring", [128, NS * SW], f32).ap()
    red4 = nc.alloc_sbuf_tensor("red4", [128, G], f32).ap()
    y2 = nc.alloc_sbuf_tensor("y2", [128, G], f32).ap()
    u2 = nc.alloc_sbuf_tensor("u2", [128, G], f32).ap()
    q2 = nc.alloc_sbuf_tensor("q2", [128, G], f32).ap()
    s2t = nc.alloc_sbuf_tensor("s2t", [128, G], f32).ap()
    accA = nc.alloc_sbuf_tensor("accA", [128, G], f32).ap()
    accB = nc.alloc_sbuf_tensor("accB", [128, G], f32).ap()
    acc_pp = [accA, accB]
    S_pp = [S0, S1]

    NX = 16 if t_steps % 16 == 0 else 1
    xchunk = t_steps // NX

    with (
        nc.semaphore("dma_sem") as dma_sem,
        nc.semaphore("d2g") as d2g,
        nc.semaphore("g2d") as g2d,
        nc.semaphore("g_done") as g_done,
        nc.Block() as block,
    ):
        sem_x = [nc.semaphore(f"sem_x{k}").__enter__() for k in range(NX)]

        @block.sync
        def _(sync):
            sync.dma_start(out=wc0[:], in_=wc0b[:]).then_inc(dma_sem, 16)
            sync.dma_start(out=wc1[:], in_=wc1b[:]).then_inc(dma_sem, 16)
            sync.dma_start(out=w2h[:], in_=w2hb[:]).then_inc(dma_sem, 16)
            sync.dma_start(out=k2[:], in_=k2b[:]).then_inc(dma_sem, 16)
            sync.dma_start(out=b1h[:], in_=b1hb[:]).then_inc(dma_sem, 16)
            for k in range(NX):
                for g in range(G):
                    sync.dma_start(
                        out=x_sbuf[
                            :,
                            g * xlen + k * xchunk * I : g * xlen
                            + (k + 1) * xchunk * I,
                        ],
                        in_=xs[
                            g * 128 : (g + 1) * 128,
                            k * xchunk * I : (k + 1) * xchunk * I,
                        ],
                    ).then_inc(sem_x[k], 16)
            sync.wait_ge(g_done, 1)
            sync.dma_start(out=out[:, :], in_=acc_pp[(t_steps - 1) % 2][:]).then_inc(
                dma_sem, 16
            )
            sync.wait_ge(dma_sem, 16 * 6)

        def scan_slot(t):
            base = (t % NS) * SW
            return (
                scanring[:, base + 1 : base + FW + 1],  # scan output
                scanring[:, base + H : base + FW + 1 : H],  # hi taps
                scanring[:, base : base + FW : H],  # lo taps
            )

        @block.vector
        def _(vector):
            vector.memset(S_pp[0][:], 0.0)
            vector.memset(scanring[:], 0.0)
            vector.memset(y2[:], 0.0)
            vector.memset(acc_pp[0][:], 0.0)
            vector.memset(acc_pp[1][:], 0.0)
            vector.wait_ge(dma_sem, 16 * 5)  # weight tiles
            for t in range(t_steps):
                src = S_pp[t % 2]
                dst = S_pp[1 - t % 2]
                if t % xchunk == 0:
                    vector.wait_ge(sem_x[t // xchunk], 16 * G)
                if t % 4 == 0 and t >= 8:
                    vector.wait_ge(g2d, t // 4 - 1)
                for g in range(G):
                    col = g * xlen + I * t
                    vector._custom_dve(
                        OP_FMA2,
                        out=cbuf[:, g * H : (g + 1) * H],
                        in0=wc0[:],
                        in1=wc1[:],
                        s0=x_sbuf[:, col : col + 1],
                        s1=x_sbuf[:, col + 1 : col + 2],
                    )
                if has_b1:
                    vector.tensor_tensor(
                        out=cbuf[:], in0=cbuf[:], in1=b1h[:], op=A.add
                    )
                vector._custom_dve(
                    OP_LIF1, out=dst[:], in0=src[:], in1=cbuf[:], s0=0.5
                )
                sout, _, _ = scan_slot(t)
                vector._custom_dve(
                    OP_SDS, out=sout, in0=dst[:], in1=w2h[:]
                ).then_inc(d2g, 1)

        @block.gpsimd
        def _(gpsimd):
            for t in range(t_steps):
                gpsimd.wait_ge(d2g, t + 1)
                _, hi, lo = scan_slot(t)
                gpsimd.tensor_tensor(out=red4[:], in0=hi, in1=lo, op=A.subtract)
                gpsimd.tensor_tensor(out=u2[:], in0=red4[:], in1=y2[:], op=A.add)
                if has_b2:
                    gpsimd.tensor_scalar(u2[:], u2[:], k2[:], None, A.add)
                if t >= decision_start:
                    gpsimd.tensor_scalar(s2t[:], u2[:], 1.0, None, A.is_ge)
                    gpsimd.tensor_tensor(
                        out=acc_pp[t % 2][:],
                        in0=acc_pp[1 - t % 2][:],
                        in1=s2t[:],
                        op=A.add,
                    )
                gpsimd.tensor_scalar(q2[:], u2[:], 1.0, 0.5, A.is_lt, A.mult)
                ins = gpsimd.tensor_tensor(out=y2[:], in0=u2[:], in1=q2[:], op=A.mult)
                if t % 4 == 3:
                    ins.then_inc(g2d, 1)
            gpsimd.tensor_scalar(q2[:], q2[:], 1.0, None, A.mult).then_inc(g_done, 1)

    mybir.codegen_inst_isa_subclasses(nc)
    return nc


def _host_tiles(W1, b1, W2, b2):
    wc0b = np.tile((W1[:, 0] * 0.5).astype(np.float32)[None, :], (128, 1))
    wc1b = np.tile((W1[:, 1] * 0.5).astype(np.float32)[None, :], (128, 1))
    w2hb = np.tile((W2[0, :] * 0.5).astype(np.float32)[None, :], (128, G))
    k2b = np.full((128, 1), 0.5 * float(b2[0]), np.float32)
    b1hb = np.tile((b1 * 0.5).astype(np.float32)[None, :], (128, G))
    return wc0b, wc1b, w2hb, k2b, b1hb


def _kernel_fallback(x, W1, b1, W2, b2):
    from concourse.bass_utils import run_bass_kernel_spmd

    has_b1 = bool(np.any(np.asarray(b1) != 0))
    has_b2 = bool(np.any(np.asarray(b2) != 0))
    key = ("nc", T, has_b1, has_b2)
    if key not in _cache:
        _cache[key] = build_nc(T, has_b1=has_b1, has_b2=has_b2)
    nc = _cache[key]

    wc0b, wc1b, w2hb, k2b, b1hb = _host_tiles(
        np.asarray(W1), np.asarray(b1), np.asarray(W2), np.asarray(b2)
    )
    x = np.ascontiguousarray(np.asarray(x, np.float32))
    in_maps = []
    for c in range(N_CORES):
        shard = x[c * B_CORE : (c + 1) * B_CORE].reshape(B_CORE, T * I)
        in_maps.append(
            {
                "xs": shard,
                "wc0b": wc0b,
                "wc1b": wc1b,
                "w2hb": w2hb,
                "k2b": k2b,
                "b1hb": b1hb,
            }
        )

    res = run_bass_kernel_spmd(nc, in_maps, list(range(N_CORES)))
    outs = [
        np.asarray(res.results[c]["out"]).T.reshape(B_CORE) for c in range(N_CORES)
    ]
    return np.concatenate(outs).reshape(B, 1).astype(np.float32)


def kernel(x, W1, b1, W2, b2):
    from concourse.bass_utils import run_bass_kernel_spmd

    x = np.ascontiguousarray(np.asarray(x, np.float32))
    W1 = np.asarray(W1, np.float32)
    b1 = np.asarray(b1, np.float32)
    W2 = np.asarray(W2, np.float32)
    b2 = np.asarray(b2, np.float32)

    live = _fast_envelope(x, W1, b1, W2, b2)
    if live is None:
        return _kernel_fallback(x, W1, b1, W2, b2)

    key = ("fast", len(live))
    if key not in _cache:
        _cache[key] = build_fast(len(live))
    nc = _cache[key]

    in_maps = _fast_inputs(x, W1, W2, live)
    res = run_bass_kernel_spmd(nc, in_maps, list(range(N_CORES)))
    total = np.zeros((128, BGF), np.float64)
    for c in range(N_CORES):
        total += np.asarray(res.results[c]["out"], np.float64)
    # counts[p, bg] holds batch row bg*128 + p
    return total.T.reshape(B, 1).astype(np.float32)


# revision 10
# speedup vs baseline: 7.1220x; 7.1220x over previous
"""Trainium2 Bass kernel: two-layer LIF spiking network scan.

Model (per timestep t, batch row b):
    h1 = x_t @ W1.T + b1            # [B, 32]
    v1 = v1 + (h1 - v1)/2           # tau = 2
    s1 = (v1 >= 1);  v1 *= (1-s1)   # hard reset
    h2 = s1 @ W2.T + b2             # [B, 1]
    v2 = v2 + (h2 - v2)/2
    s2 = (v2 >= 1);  v2 *= (1-s2)
    out = sum of s2 over t in [T - T//4, T)

Fast path (time-sharded, weight-specialized):
  * Only neurons h with max(W1[h,0],0)+max(W1[h,1],0) >= VTH can ever spike
    (x in [0,1]); the rest are dead for any input and are pruned exactly.
  * If sum_h max(W2[0,h],0) over live h < 1, layer 2 provably never resets,
    so v2 is a pure EMA: v2_t = 0.5 v2_{t-1} + 0.5 h2_t.
  * The LIF decay (tau=2) forgets exponentially: starting a window K steps
    early from v=0 reproduces the true state to ~2^-K.  With K=64 the scheme
    is exact far below fp32 noise, so the 8 cores each own a 512-step time
    window of the FULL batch (g0 = c*512 - 64) instead of a batch shard.
  * Engine split per core: PE computes c = 0.5*h1 for live neurons via
    block-diagonal matmuls (bf16, one [64,128]x[64,160] per step); ACT stages
    PSUM->SBUF (bf16); DVE runs the serial LIF step op plus chunked spike
    contributions and the v2 EMA scan; ACT counts decision-window spikes via
    a Sign-accumulate.  Host sums the two cores whose windows lie in the
    decision region.

Anything outside the envelope (weights with >8 live neurons, possible layer-2
resets, x outside [0,1], nonzero biases) falls back to the original
batch-parallel per-step kernel below, which handles the general case.
"""

import numpy as np

B, T, I, H, O = 4096, 4096, 2, 32, 1
N_CORES = 8
B_CORE = B // N_CORES          # 512 (fallback path)
G = B_CORE // 128              # 4 groups (fallback path)
TAU_INV = 0.5
VTH = 1.0

_cache = {}


# ----------------------------------------------------------------- custom ops
def _register_custom_ops():
    """Register our custom DVE ops in the process-global registry (idempotent)."""
    import concourse.dve_ops as dve_ops_mod
    from concourse.dve_ops import DveOp
    from concourse.dve_spec import (
        Spec, Src0, Src1, C0, C1, C2, Zero, One,
        select, eq, lower, AluOp, scan, _has_src1,
    )
    from concourse.dve_uop import DveOpSpec

    if "ANT_SNN_FMA2" in dve_ops_mod._SUB_OPCODE_FOR_NAME:
        return

    def _ref_fma2(in0, in1, s0, s1, imm2):
        return (in0 * s0 + in1 * s1).astype(np.float32)

    def _ref_lif1(in0, in1, s0, s1, imm2):
        # state is the pre-reset potential u: u' = (u<1) ? 0.5u + c : c
        return np.where(
            in0 < 1.0, (in0 * np.float32(0.5)) + in1, in1
        ).astype(np.float32)

    def _ref_sds(in0, in1, s0, s1, imm2):
        # prefix sums of (u >= 1) * w2h along the free dim
        contrib = np.where(in0 < 1.0, np.float32(0.0), in1)
        return np.cumsum(contrib.astype(np.float32), axis=-1, dtype=np.float32)

    specs = [
        ("ANT_SNN_FMA2", Spec(body=Src0 * C0 + Src1 * C1, reference=_ref_fma2)),
        (
            "ANT_SNN_LIF1",
            Spec(
                body=select(Src0 < One, Src0 * C0 + Src1, Src1),
                reference=_ref_lif1,
            ),
        ),
        (
            "ANT_SNN_SDS",
            Spec(
                body=scan(AluOp.ADD, select(Src0 < One, Zero, Src1)),
                reference=_ref_sds,
            ),
        ),
    ]

    ops = {}
    for name, spec in specs:
        row = 1 + len(dve_ops_mod.OPS)
        sha = {}
        for ver in ("v3", "v4"):
            try:
                s = DveOpSpec(
                    name=name,
                    opcode=row,
                    uops=lower(spec, ver=ver),
                    rd1_en=_has_src1(spec),
                )
                sha[ver] = s.sha(ver)
            except Exception:
                pass
        op = DveOp(name, spec, subdim=False, uops_sha=sha)
        dve_ops_mod.OPS.append(op)
        dve_ops_mod.CUSTOM_DVE_SPECS[name] = spec
        dve_ops_mod._SUB_OPCODE_FOR_NAME[name] = row
        ops[name] = op
    return ops


def _get_ops():
    import concourse.dve_ops as dve_ops_mod

    _register_custom_ops()
    by_name = {op.name: op for op in dve_ops_mod.OPS}
    return (
        by_name["ANT_SNN_FMA2"],
        by_name["ANT_SNN_LIF1"],
        by_name["ANT_SNN_SDS"],
    )


# =================================================================== fast path
WARM = 64                     # warmup steps before each core's 512-step window
WS = 512 + WARM               # steps simulated per core
BGF = 32                      # batch groups of 128 over the FULL batch


def build_fast(HA):
    """Per-core Bass program for the time-sharded fast path.

    HA = number of live hidden neurons (spb = PSUM steps per bank must be >=1).
    """
    import concourse.bass as bass
    import concourse.mybir as mybir

    _, OP_LIF, _ = _get_ops()
    A = mybir.AluOpType
    f32 = mybir.dt.float32
    bf16 = mybir.dt.bfloat16
    Sign = mybir.ActivationFunctionType.Sign

    NW = HA * BGF                      # matmul N / per-step state width
    spb = min(4, 512 // NW)            # matmul outputs per PSUM bank
    spp = 2 * spb                      # steps per ACT copy pair (two banks)
    CH = 32 if NW >= 288 else 48       # d-chain chunk (SBUF pressure at big NW)
    NCH = WS // CH
    RING = 2 * CH                      # uring/cring slots
    assert RING % spp == 0 and WS % spp == 0 and WS % CH == 0

    nc = bass.Bass(detect_race_conditions=False)

    xstat = nc.declare_dram_parameter("xstat", [2 * BGF, WS * 128], bf16, isOutput=False)
    wmov = nc.declare_dram_parameter("wmov", [2 * BGF, NW], bf16, isOutput=False)
    w2s = nc.declare_dram_parameter("w2s", [128, HA], f32, isOutput=False)
    wselh = nc.declare_dram_parameter("wselh", [128, 1], f32, isOutput=False)
    out = nc.declare_dram_parameter("out", [128, BGF], f32, isOutput=True)

    XCH = CH * 128
    xs_sb = nc.alloc_sbuf_tensor("xs_sb", [2 * BGF, 2, XCH], bf16)
    wmov_sb = nc.alloc_sbuf_tensor("wmov_sb", [2 * BGF, NW], bf16)
    w2s_sb = nc.alloc_sbuf_tensor("w2s_sb", [128, HA], f32)
    wselh_sb = nc.alloc_sbuf_tensor("wselh_sb", [128, 1], f32)
    uring = nc.alloc_sbuf_tensor("uring", [128, RING, NW], bf16)
    # cring rows mirror the PSUM bank pairs so one ACT copy's in/out shapes
    # match: [pair_slot, bank_of_pair, spb*NW]
    cring = nc.alloc_sbuf_tensor(
        "cring", [128, RING // (2 * spb), 2, spb * NW], bf16
    )
    dA = nc.alloc_sbuf_tensor("dA", [128, CH, BGF], bf16)
    dB = nc.alloc_sbuf_tensor("dB", [128, CH, BGF], bf16)
    k2r = nc.alloc_sbuf_tensor("k2r", [128, WS, BGF], bf16)
    u2r = nc.alloc_sbuf_tensor("u2r", [128, BGF, WS], bf16)
    halves = nc.alloc_sbuf_tensor("halves", [128, WS], bf16)
    nbias = nc.alloc_sbuf_tensor("nbias", [128, 1], f32)
    sgn = nc.alloc_sbuf_tensor("sgn", [128, 512], bf16)
    accs = nc.alloc_sbuf_tensor("accs", [128, BGF], f32)
    counts = nc.alloc_sbuf_tensor("counts", [128, BGF], f32)
    psum = nc.alloc_psum_tensor("ps", [128, 8, 512], f32)

    NPAIR = WS // spp                  # ACT copy pairs

    with (
        nc.semaphore("s_w") as s_w,
        nc.semaphore("s_pe") as s_pe,
        nc.semaphore("s_act") as s_act,
        nc.semaphore("s_dvec") as s_dvec,
        nc.semaphore("s_step") as s_step,
        nc.semaphore("s_u2") as s_u2,
        nc.semaphore("s_cnt") as s_cnt,
        nc.semaphore("s_fix") as s_fix,
        nc.semaphore("s_out") as s_out,
        nc.Block() as block,
    ):
        sem_x = [nc.semaphore(f"sx{k}").__enter__() for k in range(NCH)]

        @block.sync
        def _(sync):
            sync.dma_start(out=wmov_sb.ap()[:, :], in_=wmov[:, :]).then_inc(s_w, 16)
            sync.dma_start(out=w2s_sb.ap()[:, :], in_=w2s[:, :]).then_inc(s_w, 16)
            sync.dma_start(out=wselh_sb.ap()[:, :], in_=wselh[:, :]).then_inc(s_w, 16)
            for k in range(NCH):
                if k >= 2:
                    # xs_sb slot reuse: PE must be done with chunk k-2
                    sync.wait_ge(s_pe, (k - 1) * (CH // spb))
                sync.dma_start(
                    out=xs_sb.ap()[:, k % 2, :],
                    in_=xstat[:, k * XCH:(k + 1) * XCH],
                ).then_inc(sem_x[k], 16)
            sync.wait_ge(s_fix, 1)
            sync.dma_start(out=out[:, :], in_=counts.ap()[:, :]).then_inc(s_out, 16)
            sync.wait_ge(s_out, 16)

        @block.tensor
        def _(tensor):
            tensor.wait_ge(s_w, 16)
            for t in range(WS):
                k = t // CH
                if t % CH == 0:
                    tensor.wait_ge(sem_x[k], 16)
                g = t // spb                      # bank group index
                if t % spb == 0 and g >= 8:
                    # bank reuse: ACT pair (g-8)//2 must have drained it
                    tensor.wait_ge(s_act, (g - 8) // 2 + 1)
                ins = tensor.matmul(
                    psum.ap()[:, g % 8, (t % spb) * NW:(t % spb) * NW + NW],
                    xs_sb.ap()[:, k % 2, (t % CH) * 128:(t % CH) * 128 + 128],
                    wmov_sb.ap()[:, :],
                    True, True,
                )
                if t % spb == spb - 1:
                    ins.then_inc(s_pe, 1)

        @block.scalar
        def _(scalar):
            for p in range(NPAIR):
                t0 = p * spp
                kc = t0 // CH
                scalar.wait_ge(s_pe, 2 * p + 2)
                if kc >= 2:
                    # cring slot reuse: DVE must be done with chunk kc-2
                    scalar.wait_ge(s_dvec, kc - 1)
                scalar.copy(
                    out=cring.ap()[:, (t0 // spp) % (RING // spp), :, :],
                    in_=psum.ap()[:, (2 * p) % 8:(2 * p) % 8 + 2, 0:spb * NW],
                ).then_inc(s_act, 1)
            # --- decision-window spike count (per batch group) ---
            scalar.wait_ge(s_u2, 1)
            for gidx in range(BGF):
                ins = scalar.activation(
                    out=sgn.ap()[:, :],
                    in_=u2r.ap()[:, gidx, WARM:WS],
                    func=Sign,
                    bias=nbias.ap()[:, :],
                    scale=1.0,
                    accum_out=accs.ap()[:, gidx:gidx + 1],
                )
                if gidx == BGF - 1:
                    ins.then_inc(s_cnt, 1)

        @block.vector
        def _(vector):
            vector.memset(uring.ap()[:, RING - 1, :], 0.0)
            vector.memset(halves.ap()[:, :], 0.5)
            vector.memset(nbias.ap()[:, :], -0.998)
            vector.wait_ge(s_w, 48)
            for t in range(WS):
                if t % spp == 0:
                    vector.wait_ge(s_act, t // spp + 1)
                # serial LIF step over all live chains; the then_inc flushes
                # the write so the next op's read is fresh in the interp.
                ins = vector._custom_dve(
                    OP_LIF,
                    out=uring.ap()[:, t % RING, :],
                    in0=uring.ap()[:, (t - 1) % RING, :],
                    in1=cring.ap()[
                        :, (t // spp) % (RING // spp), (t // spb) % 2,
                        (t % spb) * NW:(t % spb) * NW + NW,
                    ],
                    s0=0.5,
                )
                if t % CH == CH - 1:
                    # cring chunk fully consumed -> ACT may reuse the slots
                    # (this inc also flushes the state write for the d-chain)
                    ins.then_inc(s_dvec, 1)
                    kc = t // CH
                    sl = (kc % 2) * CH
                    u3 = uring.ap()
                    tsl = slice(sl, sl + CH)
                    vector.tensor_scalar(
                        dA.ap()[:, :, :], u3[:, tsl, 0:BGF],
                        1.0, w2s_sb.ap()[:, 0:1], A.is_ge, A.mult,
                    )
                    for h in range(1, HA):
                        vector.tensor_scalar(
                            dB.ap()[:, :, :], u3[:, tsl, h * BGF:(h + 1) * BGF],
                            1.0, w2s_sb.ap()[:, h:h + 1], A.is_ge, A.mult,
                        ).then_inc(s_step, 1)
                        tgt = (
                            k2r.ap()[:, kc * CH:(kc + 1) * CH, :]
                            if h == HA - 1 else dA.ap()[:, :, :]
                        )
                        vector.tensor_tensor(
                            out=tgt, in0=dA.ap()[:, :, :], in1=dB.ap()[:, :, :],
                            op=A.add,
                        ).then_inc(s_step, 1)
                else:
                    ins.then_inc(s_step, 1)
            # --- v2 EMA per batch group (never resets: sup u2 < 1) ---
            for gidx in range(BGF):
                ins = vector.tensor_tensor_scan(
                    u2r.ap()[:, gidx, :],
                    halves.ap()[:, :],
                    k2r.ap()[:, :, gidx],
                    0.0, A.mult, A.add,
                )
                if gidx == BGF - 1:
                    ins.then_inc(s_u2, 1)
            vector.wait_ge(s_cnt, 1)
            vector.tensor_scalar(
                counts.ap()[:, :], accs.ap()[:, :], 512.0, wselh_sb.ap()[:, :],
                A.add, A.mult,
            ).then_inc(s_fix, 1)

    mybir.codegen_inst_isa_subclasses(nc)
    return nc


def _fast_envelope(x, W1, b1, W2, b2):
    """Live-neuron set + rigorous layer-2-silence check.  Returns the live
    index array, or None if the fast path's assumptions don't hold.

    A neuron h can only spike if sup_x u1 = max(W1[h,0],0)+max(W1[h,1],0)
    exceeds VTH (x in [0,1]).  After any reset it needs g_h steps to rebuild
    (u_g <= sup*(1-2^-g)), so its spike train has min gap g_h and its EMA
    contribution to u2 is at most 0.5*w2h+/(1-2^-g_h).  If the sum over live
    neurons stays below 1 with margin, layer 2 provably never resets for ANY
    input, and v2 is an exact EMA."""
    if b1.any() or b2.any():
        return None
    if x.min() < 0.0 or x.max() > 1.0:
        return None
    sup1 = np.maximum(W1[:, 0], 0) + np.maximum(W1[:, 1], 0)
    live = np.where(sup1 >= VTH - 1e-5)[0]
    if len(live) < 1 or len(live) > 16:
        return None
    g = np.ceil(-np.log2(np.maximum(
        1.0 - 1.0 / np.maximum(sup1[live].astype(np.float64), 1.0 + 1e-9), 1e-12
    )))
    g = np.maximum(g, 1.0)
    u2_sup = (0.5 * np.maximum(W2[0, live], 0) / (1.0 - 2.0 ** (-g))).sum()
    if u2_sup >= 0.95:
        return None
    return live


def _fast_inputs(x, W1, W2, live):
    import jax.numpy as jnp

    HA = len(live)
    NW = HA * BGF
    to_bf = lambda a: np.asarray(jnp.asarray(np.asarray(a, np.float32), jnp.bfloat16))

    # wmov[(bg,i), h*BGF+bg] = 0.5*W1[live[h], i]
    wmov = np.zeros((2 * BGF, NW), np.float32)
    for bg in range(BGF):
        for i in range(I):
            for h in range(HA):
                wmov[bg * 2 + i, h * BGF + bg] = 0.5 * W1[live[h], i]
    wmov = to_bf(wmov)
    w2s = np.tile((0.5 * W2[0, live])[None, :], (128, 1)).astype(np.float32)

    # xstat per core: [(bg,i), t*128+p] = x[bg*128+p, g0+t, i]
    xr = x.reshape(BGF, 128, T, I)                    # [bg, p, t, i]
    xstats = []
    for c in range(N_CORES):
        g0 = c * 512 - WARM
        arr = np.zeros((BGF, 128, WS, I), np.float32)
        lo, hi = max(g0, 0), min(g0 + WS, T)
        arr[:, :, lo - g0:hi - g0, :] = xr[:, :, lo:hi, :]
        # -> [(bg, i), t, p]
        arr = arr.transpose(0, 3, 2, 1).reshape(2 * BGF, WS * 128)
        xstats.append(to_bf(arr))

    in_maps = []
    for c in range(N_CORES):
        wsel = 0.5 if c * 512 >= max(T - T // 4, T // 2) else 0.0
        in_maps.append({
            "xstat": xstats[c],
            "wmov": wmov,
            "w2s": w2s,
            "wselh": np.full((128, 1), wsel, np.float32),
        })
    return in_maps


# ------------------------------------------------------------- fallback build
def build_nc(t_steps=T, decision_start=None, has_b1=False, has_b2=False):
    """Build the per-core Bass program (SPMD; all cores run the same NEFF)."""
    import concourse.bass as bass
    import concourse.mybir as mybir

    OP_FMA2, OP_LIF1, OP_SDS = _get_ops()
    A = mybir.AluOpType
    f32 = mybir.dt.float32

    if decision_start is None:
        decision_start = max(t_steps - t_steps // 4, t_steps // 2)

    # Same-engine RAW hazards are safe on HW (per-op DVE pipeline drain);
    # the CoreSim race detector would flag them, so turn it off.
    nc = bass.Bass(detect_race_conditions=False)

    xs = nc.declare_dram_parameter("xs", [B_CORE, t_steps * I], f32, isOutput=False)
    wc0b = nc.declare_dram_parameter("wc0b", [128, H], f32, isOutput=False)
    wc1b = nc.declare_dram_parameter("wc1b", [128, H], f32, isOutput=False)
    w2hb = nc.declare_dram_parameter("w2hb", [128, G * H], f32, isOutput=False)
    k2b = nc.declare_dram_parameter("k2b", [128, 1], f32, isOutput=False)
    b1hb = nc.declare_dram_parameter("b1hb", [128, G * H], f32, isOutput=False)
    out = nc.declare_dram_parameter("out", [128, G], f32, isOutput=True)

    xlen = t_steps * I
    FW = G * H  # 128 free width for the fused tiles

    x_sbuf = nc.alloc_sbuf_tensor("x_sbuf", [128, G * xlen], f32).ap()
    wc0 = nc.alloc_sbuf_tensor("wc0", [128, H], f32).ap()
    wc1 = nc.alloc_sbuf_tensor("wc1", [128, H], f32).ap()
    w2h = nc.alloc_sbuf_tensor("w2h", [128, FW], f32).ap()
    b1h = nc.alloc_sbuf_tensor("b1h", [128, FW], f32).ap()
    k2 = nc.alloc_sbuf_tensor("k2", [128, 1], f32).ap()
    NS = 8  # scan ring depth (DVE->gpsimd decoupling, in steps)
    SW = FW + 4  # scan slot width
    S0 = nc.alloc_sbuf_tensor("S0", [128, FW], f32).ap()
    S1 = nc.alloc_sbuf_tensor("S1", [128, FW], f32).ap()
    cbuf = nc.alloc_sbuf_tensor("cbuf", [128, FW], f32).ap()
    scanring = nc.alloc_sbuf_tensor("scanring", [128, NS * SW], f32).ap()
    red4 = nc.alloc_sbuf_tensor("red4", [128, G], f32).ap()
    y2 = nc.alloc_sbuf_tensor("y2", [128, G], f32).ap()
    u2 = nc.alloc_sbuf_tensor("u2", [128, G], f32).ap()
    q2 = nc.alloc_sbuf_tensor("q2", [128, G], f32).ap()
    s2t = nc.alloc_sbuf_tensor("s2t", [128, G], f32).ap()
    accA = nc.alloc_sbuf_tensor("accA", [128, G], f32).ap()
    accB = nc.alloc_sbuf_tensor("accB", [128, G], f32).ap()
    acc_pp = [accA, accB]
    S_pp = [S0, S1]

    NX = 16 if t_steps % 16 == 0 else 1
    xchunk = t_steps // NX

    with (
        nc.semaphore("dma_sem") as dma_sem,
        nc.semaphore("d2g") as d2g,
        nc.semaphore("g2d") as g2d,
        nc.semaphore("g_done") as g_done,
        nc.Block() as block,
    ):
        sem_x = [nc.semaphore(f"sem_x{k}").__enter__() for k in range(NX)]

        @block.sync
        def _(sync):
            sync.dma_start(out=wc0[:], in_=wc0b[:]).then_inc(dma_sem, 16)
            sync.dma_start(out=wc1[:], in_=wc1b[:]).then_inc(dma_sem, 16)
            sync.dma_start(out=w2h[:], in_=w2hb[:]).then_inc(dma_sem, 16)
            sync.dma_start(out=k2[:], in_=k2b[:]).then_inc(dma_sem, 16)
            sync.dma_start(out=b1h[:], in_=b1hb[:]).then_inc(dma_sem, 16)
            for k in range(NX):
                for g in range(G):
                    sync.dma_start(
                        out=x_sbuf[
                            :,
                            g * xlen + k * xchunk * I : g * xlen
                            + (k + 1) * xchunk * I,
                        ],
                        in_=xs[
                            g * 128 : (g + 1) * 128,
                            k * xchunk * I : (k + 1) * xchunk * I,
                        ],
                    ).then_inc(sem_x[k], 16)
            sync.wait_ge(g_done, 1)
            sync.dma_start(out=out[:, :], in_=acc_pp[(t_steps - 1) % 2][:]).then_inc(
                dma_sem, 16
            )
            sync.wait_ge(dma_sem, 16 * 6)

        def scan_slot(t):
            base = (t % NS) * SW
            return (
                scanring[:, base + 1 : base + FW + 1],  # scan output
                scanring[:, base + H : base + FW + 1 : H],  # hi taps
                scanring[:, base : base + FW : H],  # lo taps
            )

        @block.vector
        def _(vector):
            vector.memset(S_pp[0][:], 0.0)
            vector.memset(scanring[:], 0.0)
            vector.memset(y2[:], 0.0)
            vector.memset(acc_pp[0][:], 0.0)
            vector.memset(acc_pp[1][:], 0.0)
            vector.wait_ge(dma_sem, 16 * 5)  # weight tiles
            for t in range(t_steps):
                src = S_pp[t % 2]
                dst = S_pp[1 - t % 2]
                if t % xchunk == 0:
                    vector.wait_ge(sem_x[t // xchunk], 16 * G)
                if t % 4 == 0 and t >= 8:
                    vector.wait_ge(g2d, t // 4 - 1)
                for g in range(G):
                    col = g * xlen + I * t
                    vector._custom_dve(
                        OP_FMA2,
                        out=cbuf[:, g * H : (g + 1) * H],
                        in0=wc0[:],
                        in1=wc1[:],
                        s0=x_sbuf[:, col : col + 1],
                        s1=x_sbuf[:, col + 1 : col + 2],
                    )
                if has_b1:
                    vector.tensor_tensor(
                        out=cbuf[:], in0=cbuf[:], in1=b1h[:], op=A.add
                    )
                vector._custom_dve(
                    OP_LIF1, out=dst[:], in0=src[:], in1=cbuf[:], s0=0.5
                )
                sout, _, _ = scan_slot(t)
                vector._custom_dve(
                    OP_SDS, out=sout, in0=dst[:], in1=w2h[:]
                ).then_inc(d2g, 1)

        @block.gpsimd
        def _(gpsimd):
            for t in range(t_steps):
                gpsimd.wait_ge(d2g, t + 1)
                _, hi, lo = scan_slot(t)
                gpsimd.tensor_tensor(out=red4[:], in0=hi, in1=lo, op=A.subtract)
                gpsimd.tensor_tensor(out=u2[:], in0=red4[:], in1=y2[:], op=A.add)
                if has_b2:
                    gpsimd.tensor_scalar(u2[:], u2[:], k2[:], None, A.add)
                if t >= decision_start:
                    gpsimd.tensor_scalar(s2t[:], u2[:], 1.0, None, A.is_ge)
                    gpsimd.tensor_tensor(
                        out=acc_pp[t % 2][:],
                        in0=acc_pp[1 - t % 2][:],
                        in1=s2t[:],
                        op=A.add,
                    )
                gpsimd.tensor_scalar(q2[:], u2[:], 1.0, 0.5, A.is_lt, A.mult)
                ins = gpsimd.tensor_tensor(out=y2[:], in0=u2[:], in1=q2[:], op=A.mult)
                if t % 4 == 3:
                    ins.then_inc(g2d, 1)
            gpsimd.tensor_scalar(q2[:], q2[:], 1.0, None, A.mult).then_inc(g_done, 1)

    mybir.codegen_inst_isa_subclasses(nc)
    return nc


def _host_tiles(W1, b1, W2, b2):
    wc0b = np.tile((W1[:, 0] * 0.5).astype(np.float32)[None, :], (128, 1))
    wc1b = np.tile((W1[:, 1] * 0.5).astype(np.float32)[None, :], (128, 1))
    w2hb = np.tile((W2[0, :] * 0.5).astype(np.float32)[None, :], (128, G))
    k2b = np.full((128, 1), 0.5 * float(b2[0]), np.float32)
    b1hb = np.tile((b1 * 0.5).astype(np.float32)[None, :], (128, G))
    return wc0b, wc1b, w2hb, k2b, b1hb


def _kernel_fallback(x, W1, b1, W2, b2):
    from concourse.bass_utils import run_bass_kernel_spmd

    has_b1 = bool(np.any(np.asarray(b1) != 0))
    has_b2 = bool(np.any(np.asarray(b2) != 0))
    key = ("nc", T, has_b1, has_b2)
    if key not in _cache:
        _cache[key] = build_nc(T, has_b1=has_b1, has_b2=has_b2)
    nc = _cache[key]

    wc0b, wc1b, w2hb, k2b, b1hb = _host_tiles(
        np.asarray(W1), np.asarray(b1), np.asarray(W2), np.asarray(b2)
    )
    x = np.ascontiguousarray(np.asarray(x, np.float32))
    in_maps = []
    for c in range(N_CORES):
        shard = x[c * B_CORE : (c + 1) * B_CORE].reshape(B_CORE, T * I)
        in_maps.append(
            {
                "xs": shard,
                "wc0b": wc0b,
                "wc1b": wc1b,
                "w2hb": w2hb,
                "k2b": k2b,
                "b1hb": b1hb,
            }
        )

    res = run_bass_kernel_spmd(nc, in_maps, list(range(N_CORES)))
    outs = [
        np.asarray(res.results[c]["out"]).T.reshape(B_CORE) for c in range(N_CORES)
    ]
    return np.concatenate(outs).reshape(B, 1).astype(np.float32)


def kernel(x, W1, b1, W2, b2):
    from concourse.bass_utils import run_bass_kernel_spmd

    x = np.ascontiguousarray(np.asarray(x, np.float32))
    W1 = np.asarray(W1, np.float32)
    b1 = np.asarray(b1, np.float32)
    W2 = np.asarray(W2, np.float32)
    b2 = np.asarray(b2, np.float32)

    live = _fast_envelope(x, W1, b1, W2, b2)
    if live is None:
        return _kernel_fallback(x, W1, b1, W2, b2)

    key = ("fast", len(live))
    if key not in _cache:
        _cache[key] = build_fast(len(live))
    nc = _cache[key]

    in_maps = _fast_inputs(x, W1, W2, live)
    res = run_bass_kernel_spmd(nc, in_maps, list(range(N_CORES)))
    total = np.zeros((128, BGF), np.float64)
    for c in range(N_CORES):
        total += np.asarray(res.results[c]["out"], np.float64)
    # counts[p, bg] holds batch row bg*128 + p
    return total.T.reshape(B, 1).astype(np.float32)
